# revision 9
# baseline (speedup 1.0000x reference)
"""BezierAlign distributed Trainium2 kernel (fp16, pair-row gather table).

Contract: kernel(input, beziers) -> [256, 256, 16, 64] f32, computed on the
8 NeuronCores. Host side only shards/routes/reassembles:
  - ROIs are routed to cores so each core's 32 ROIs live in <= 2 adjacent
    batches (sharding_hint: route ROIs by batch / shard by ROI).
  - The feature map is resharded per core as an fp16 NHWC pixel-PAIR table:
    row r = [channels of pixel r, channels of pixel r+W] (1 KiB), so a
    single 2 KiB gather descriptor starting at row (y_low*W + x_low) fetches
    all 4 bilinear taps (y/y+1 x x/x+1) of a bin.

Per-core device program (SPMD, identical on all 8 cores):
  - bezier control points -> sample coords X, Y for all 32x1024 bins via
    PE matmuls against a constant basis matrix (layout [bin%128, g*32+n]),
    all in f32 (indices stay exact).
  - bilinear indices/weights via DVE elementwise ops; gather indices folded
    to dma_gather's 16-partition-wrapped int16 layout (identity-slice PE
    matmuls + permuted copies).
  - per ROI: ONE 1024-index dma_gather pulls 2 MiB of fp16 tap data;
    8 groups x 8 fp16 matmuls against diag(weight) matrices apply the 4
    bilinear terms and transpose [bin, ch] -> [ch, bin] into one f32 PSUM
    tile; 2 Act copies assemble fp16 [256ch, 1024bin]; 2 HWDGE DMAs write
    the ROI's NCHW output (fp16, upcast on host).
"""

import numpy as np

B, C, H, W = 8, 256, 128, 128
N_ROIS = 256
PH, PW = 16, 64
NB = PH * PW              # 1024 bins per ROI
NCORES = 8
R = 32                    # ROIs per core
SCALE = 0.25
PIX = H * W               # 16384 pixel rows per batch
TROWS = 2 * PIX           # table rows addressable by int16 idx (32768)
PADROWS = 192             # zero rows appended (x+1 overrun + idx clamp)
BUFROWS = TROWS + PADROWS
EW = 2 * C                # elems per pair-row (y and y+1 channel vectors)

_cache = {}


def _basis_const():
    """M[k, t]: X[n, t] = sum_k ctrl_x[n, k] * M[k, t], t = i*64 + j."""
    t = np.arange(NB)
    i, j = t // PW, t % PW
    u = j.astype(np.float64) / PW
    v = i.astype(np.float64) / PH
    co = [1.0, 3.0, 3.0, 1.0]
    M = np.zeros((8, NB), np.float64)
    for k in range(4):
        bern = co[k] * u**k * (1.0 - u) ** (3 - k)
        M[k] = SCALE * bern * (1.0 - v)       # top curve
        M[4 + k] = SCALE * bern * v           # bottom curve
    # Permute columns to the on-device (g, p) layout. Bin-at-position map
    # t(p, g) = (p%16)*64 + g*8 + p//16 makes DMA engine r (= position%16)
    # walk output row r along the curve -> near-contiguous gather addresses.
    gg = np.arange(8)[:, None]
    pp = np.arange(128)[None, :]
    tcol = (pp % 16) * 64 + gg * 8 + pp // 16
    return M[:, tcol.reshape(-1)].astype(np.float32)


def _build_program(rep=1):
    """rep>1 wraps the main ROI loop in a hardware repeat loop (benchmarking
    only — output is rewritten identically each iteration)."""
    import contextlib
    import concourse.bass as bass
    import concourse.bacc as bacc
    import concourse.tile as tile
    from concourse import mybir

    f32 = mybir.dt.float32
    f16 = mybir.dt.float16
    Alu = mybir.AluOpType
    Act = mybir.ActivationFunctionType

    nc = bacc.Bacc("TRN2", target_bir_lowering=False, debug=False)
    feat = nc.dram_tensor("feat", [BUFROWS, EW], f16, kind="ExternalInput")
    bez = nc.dram_tensor("bez", [R, 17], f32, kind="ExternalInput")
    mconst = nc.dram_tensor("mconst", [8, NB], f32, kind="ExternalInput")
    eye_d = nc.dram_tensor("eye", [128, 128], f32, kind="ExternalInput")
    eye16_d = nc.dram_tensor("eye16", [128, 128], f16, kind="ExternalInput")
    id32_d = nc.dram_tensor("id32", [32, 32], f32, kind="ExternalInput")
    c16k_d = nc.dram_tensor("c16k", [1, 128], f32, kind="ExternalInput")
    out_d = nc.dram_tensor("out", [R, C, PH, PW], f16, kind="ExternalOutput")

    # gather table view: idx unit = 1 pair-row (512 f16), elem = 2 pair-rows
    gather_src = bass.AP(feat[:].tensor, 0, [[EW, TROWS], [1, 2 * EW]])

    with tile.TileContext(nc) as tc:
        with (
            tc.tile_pool(name="const", bufs=1) as cpool,
            tc.tile_pool(name="work", bufs=1) as wpool,
        ):
            eye = cpool.tile([128, 128], f32)
            nc.sync.dma_start(eye[:], eye_d[:])
            eye16 = cpool.tile([128, 128], f16)
            nc.sync.dma_start(eye16[:], eye16_d[:])
            m_sb = cpool.tile([8, NB], f32)
            nc.sync.dma_start(m_sb[:], mconst[:])
            id32 = cpool.tile([32, 32], f32)
            nc.sync.dma_start(id32[:], id32_d[:])
            c16k = cpool.tile([1, 128], f32)
            nc.sync.dma_start(c16k[:], c16k_d[:])
            bez_sb = cpool.tile([R, 17], f32)
            nc.sync.dma_start(bez_sb[:], bez[:])

            w00 = wpool.tile([128, 256], f32)
            w01 = wpool.tile([128, 256], f32)
            w10 = wpool.tile([128, 256], f32)
            w11 = wpool.tile([128, 256], f32)
            widx = wpool.tile([128, R * 64], mybir.dt.int16)

            with (
                tc.tile_pool(name="setup", bufs=1) as spool,
                tc.tile_pool(name="psetup", bufs=1, space="PSUM") as pspool,
            ):
                # --- control points -> per-bin coords ------------------
                p_sep = spool.tile([R, 17], f32)
                nc.vector.tensor_copy(p_sep[:, 0:8], bez_sb[:, 1:17:2])
                nc.vector.tensor_copy(p_sep[:, 8:16], bez_sb[:, 2:17:2])
                nc.vector.tensor_copy(p_sep[:, 16:17], bez_sb[:, 0:1])
                pt_ps = pspool.tile([8, 3 * 32], f32)
                nc.tensor.transpose(out=pt_ps[0:8, 0:32], in_=p_sep[:, 0:8],
                                    identity=id32[:])
                nc.tensor.transpose(out=pt_ps[0:8, 32:64], in_=p_sep[:, 8:16],
                                    identity=id32[:])
                nc.tensor.transpose(out=pt_ps[0:1, 64:96], in_=p_sep[:, 16:17],
                                    identity=id32[:])
                pt = spool.tile([8, 3 * 32], f32)
                nc.vector.tensor_copy(pt[:, 0:64], pt_ps[0:8, 0:64])
                nc.vector.tensor_copy(pt[0:1, 64:96], pt_ps[0:1, 64:96])

                ps_x = pspool.tile([128, 256], f32)
                ps_y = pspool.tile([128, 256], f32)
                ps_b = pspool.tile([128, 256], f32)
                for g in range(8):
                    sl = slice(g * 32, (g + 1) * 32)
                    nc.tensor.matmul(out=ps_x[:, sl], lhsT=m_sb[:, g * 128:(g + 1) * 128],
                                     rhs=pt[0:8, 0:32], start=True, stop=True)
                    nc.tensor.matmul(out=ps_y[:, sl], lhsT=m_sb[:, g * 128:(g + 1) * 128],
                                     rhs=pt[0:8, 32:64], start=True, stop=True)
                    nc.tensor.matmul(out=ps_b[:, sl], lhsT=c16k[:],
                                     rhs=pt[0:1, 64:96], start=True, stop=True)

                # --- bilinear indices + weights ------------------------
                def T(name):
                    return spool.tile([128, 256], f32, name=name)

                xs, xl, lx, hx = T("xs"), T("xl"), T("lx"), T("hx")
                ys, yl, ly, hy = T("ys"), T("yl"), T("ly"), T("hy")
                tmp, val, tv = T("tmp"), T("val"), T("tv")
                ixf = T("ixf")

                v = nc.vector
                MAGIC = 12582912.0  # 1.5 * 2**23: (x+M)-M rounds x to nearest
                v.tensor_scalar(xs[:], ps_x[:], 0.0, None, Alu.max)
                v.tensor_scalar(xl[:], xs[:], MAGIC, -MAGIC, Alu.add, Alu.add)
                v.tensor_tensor(tmp[:], xl[:], xs[:], Alu.is_gt)
                v.tensor_tensor(xl[:], xl[:], tmp[:], Alu.subtract)
                v.tensor_scalar(xl[:], xl[:], float(W - 1), None, Alu.min)
                v.tensor_tensor(lx[:], xs[:], xl[:], Alu.subtract)
                v.tensor_scalar(tmp[:], xl[:], float(W - 1), None, Alu.is_lt)
                v.tensor_tensor(lx[:], lx[:], tmp[:], Alu.mult)
                v.tensor_scalar(hx[:], lx[:], 1.0, -1.0, Alu.subtract, Alu.mult)

                v.tensor_scalar(ys[:], ps_y[:], 0.0, None, Alu.max)
                v.tensor_scalar(yl[:], ys[:], MAGIC, -MAGIC, Alu.add, Alu.add)
                v.tensor_tensor(tmp[:], yl[:], ys[:], Alu.is_gt)
                v.tensor_tensor(yl[:], yl[:], tmp[:], Alu.subtract)
                v.tensor_scalar(yl[:], yl[:], float(H - 1), None, Alu.min)
                v.tensor_tensor(ly[:], ys[:], yl[:], Alu.subtract)
                v.tensor_scalar(tmp[:], yl[:], float(H - 1), None, Alu.is_lt)
                v.tensor_tensor(ly[:], ly[:], tmp[:], Alu.mult)
                v.tensor_scalar(hy[:], ly[:], 1.0, -1.0, Alu.subtract, Alu.mult)

                v.tensor_scalar(val[:], ps_x[:], float(W), None, Alu.is_lt)
                v.tensor_scalar(tv[:], ps_x[:], -1.0, None, Alu.is_gt)
                v.tensor_tensor(val[:], val[:], tv[:], Alu.mult)
                v.tensor_scalar(tv[:], ps_y[:], float(H), None, Alu.is_lt)
                v.tensor_tensor(val[:], val[:], tv[:], Alu.mult)
                v.tensor_scalar(tv[:], ps_y[:], -1.0, None, Alu.is_gt)
                v.tensor_tensor(val[:], val[:], tv[:], Alu.mult)
                v.tensor_tensor(hy[:], hy[:], val[:], Alu.mult)
                v.tensor_tensor(ly[:], ly[:], val[:], Alu.mult)

                v.tensor_tensor(w00[:], hy[:], hx[:], Alu.mult)
                v.tensor_tensor(w01[:], hy[:], lx[:], Alu.mult)
                v.tensor_tensor(w10[:], ly[:], hx[:], Alu.mult)
                v.tensor_tensor(w11[:], ly[:], lx[:], Alu.mult)

                v.tensor_scalar(ixf[:], yl[:], float(W), None, Alu.mult)
                v.tensor_tensor(ixf[:], ixf[:], xl[:], Alu.add)
                v.tensor_tensor(ixf[:], ixf[:], ps_b[:], Alu.add)

                # --- fold idx to dma_gather wrapped layout -------------
                # widx[r, n*64 + g*8 + q] = ix[q*16+r, g*32+n]
                fold = pspool.tile([16, 2048], f32, name="fold")
                for q in range(8):
                    nc.tensor.matmul(out=fold[:, q * 256:(q + 1) * 256],
                                     lhsT=eye[:, q * 16:(q + 1) * 16],
                                     rhs=ixf[:], start=True, stop=True)
                dst = (widx[0:16, :]
                       .rearrange("r (n g q) -> r n g q", n=R, g=8))
                perm = fold[:].rearrange("r (q g n) -> r n g q", q=8, g=8, n=32)
                nc.vector.tensor_copy(dst, perm)
                for k in range(1, 8):
                    nc.sync.dma_start(widx[16 * k:16 * (k + 1), :], widx[0:16, :])

            # --- main ROI loop ------------------------------------------
            with (
                tc.tile_pool(name="gath", bufs=3) as gpool,
                tc.tile_pool(name="stg", bufs=3) as stpool,
                tc.tile_pool(name="diag", bufs=3) as dpool,
                tc.tile_pool(name="pmain", bufs=2, space="PSUM") as ppool,
                tc.For_i(0, rep, 1) if rep > 1 else contextlib.nullcontext(),
            ):
                for n in range(R):
                    # one gather per ROI: elem = 2 pair-rows = all 4 taps,
                    # 2 KiB per descriptor. e = xoff*512 + k2*256 + ch
                    gab = gpool.tile([128, 8 * 1024], f16, name="gab")
                    nc.gpsimd.dma_gather(
                        out_ap=gab[:].rearrange("p (t e) -> p t e", e=1024),
                        in_ap=gather_src,
                        idxs_ap=widx[:, n * 64:(n + 1) * 64],
                        num_idxs=1024,
                        num_idxs_reg=1024,
                        elem_size=1024,
                        elem_step=512,
                    )
                    stage = stpool.tile([128, 2048], f16, name="stage")
                    po = ppool.tile([128, 2048], f32, name="po")
                    for g in range(8):
                        col = g * 32 + n
                        d00 = dpool.tile([128, 128], f16, name="d00")
                        d01 = dpool.tile([128, 128], f16, name="d01")
                        d10 = dpool.tile([128, 128], f16, name="d10")
                        d11 = dpool.tile([128, 128], f16, name="d11")
                        nc.vector.tensor_scalar(d00[:], eye16[:], w00[:, col:col + 1],
                                                None, Alu.mult)
                        nc.vector.tensor_scalar(d01[:], eye16[:], w01[:, col:col + 1],
                                                None, Alu.mult)
                        nc.vector.tensor_scalar(d10[:], eye16[:], w10[:, col:col + 1],
                                                None, Alu.mult)
                        nc.vector.tensor_scalar(d11[:], eye16[:], w11[:, col:col + 1],
                                                None, Alu.mult)
                        for h in range(2):
                            osl = slice(g * 256 + h * 128, g * 256 + (h + 1) * 128)
                            base = g * 1024 + h * 128
                            nc.tensor.matmul(out=po[:, osl], rhs=d00[:],
                                             lhsT=gab[:, base:base + 128],
                                             start=True, stop=False)
                            nc.tensor.matmul(out=po[:, osl], rhs=d10[:],
                                             lhsT=gab[:, base + 256:base + 384],
                                             start=False, stop=False)
                            nc.tensor.matmul(out=po[:, osl], rhs=d01[:],
                                             lhsT=gab[:, base + 512:base + 640],
                                             start=False, stop=False)
                            nc.tensor.matmul(out=po[:, osl], rhs=d11[:],
                                             lhsT=gab[:, base + 768:base + 896],
                                             start=False, stop=True)
                    # scatter psum (g, h, p) -> stage (h, bin t(p,g)): one
                    # strided copy per channel half h, on the Act engine
                    for h in range(2):
                        src = (po[:].rearrange("c (g h2 a r) -> c h2 g a r",
                                               g=8, h2=2, a=8)[:, h])
                        dst = (stage[:, h * 1024:(h + 1) * 1024]
                               .rearrange("c (r g2 a) -> c g2 a r",
                                          r=16, g2=8, a=8))
                        nc.scalar.activation(dst, src, Act.Copy)
                    for h in range(2):
                        nc.sync.dma_start(out_d[n, h * 128:(h + 1) * 128, :, :],
                                          stage[:, h * 1024:(h + 1) * 1024])

    nc.compile()
    return nc


def _route(batch):
    """Assign ROIs to cores: sorted by batch, each core spans <=2 adjacent
    batches, <=R ROIs. Returns (ids_per_core, base_per_core)."""
    order = np.argsort(batch, kind="stable")
    n = len(order)
    ids, bases = [], []
    i = 0
    for _ in range(NCORES):
        if i >= n:
            ids.append([])
            bases.append(0)
            continue
        base = int(batch[order[i]])
        cur = []
        while i < n and len(cur) < R and int(batch[order[i]]) <= base + 1:
            cur.append(int(order[i]))
            i += 1
        ids.append(cur)
        bases.append(base)
    if i < n:
        raise RuntimeError("ROI->core routing failed (batch distribution too "
                           "skewed for 8 cores x 2 batches)")
    return ids, bases


def kernel(input, beziers):
    from concourse.bass_utils import run_bass_kernel_spmd

    input = np.asarray(input, dtype=np.float32)
    beziers = np.asarray(beziers, dtype=np.float32)

    if "nc" not in _cache:
        _cache["nc"] = _build_program()
    nc = _cache["nc"]

    batch = beziers[:, 0].astype(np.int32)
    ids, bases = _route(batch)

    nhwc = np.ascontiguousarray(
        input.transpose(0, 2, 3, 1)).reshape(B * PIX, C).astype(np.float16)
    consts = {
        "mconst": _basis_const(),
        "eye": np.eye(128, dtype=np.float32),
        "eye16": np.eye(128, dtype=np.float16),
        "id32": np.eye(32, dtype=np.float32),
        "c16k": np.full((1, 128), float(PIX), np.float32),
    }

    in_maps = []
    for c in range(NCORES):
        buf = np.zeros((BUFROWS, EW), np.float16)
        lo = bases[c] * PIX
        hi = min((bases[c] + 2) * PIX, B * PIX)
        nrow = hi - lo
        buf[:nrow, :C] = nhwc[lo:hi]
        buf[:nrow - W, C:] = nhwc[lo + W:hi]          # pixel r+W channels
        bz = np.zeros((R, 17), np.float32)
        if ids[c]:
            rows = beziers[ids[c]].copy()
            rows[:, 0] = batch[ids[c]] - bases[c]
            bz[:len(ids[c])] = rows
        in_maps.append({"feat": buf, "bez": bz, **consts})

    _cache["in_maps"] = in_maps
    res = run_bass_kernel_spmd(nc, in_maps, list(range(NCORES)))

    out = np.zeros((N_ROIS, C, PH, PW), np.float32)
    for c in range(NCORES):
        if ids[c]:
            out[ids[c]] = res.results[c]["out"][:len(ids[c])].astype(np.float32)
    return out


# revision 11
# speedup vs baseline: 1.0169x; 1.0169x over previous
"""BezierAlign distributed Trainium2 kernel (fp16, pair-row gather table).

Contract: kernel(input, beziers) -> [256, 256, 16, 64] f32, computed on the
8 NeuronCores. Host side only shards/routes/reassembles:
  - ROIs are routed to cores so each core's 32 ROIs live in <= 2 adjacent
    batches (sharding_hint: route ROIs by batch / shard by ROI).
  - The feature map is resharded per core as an fp16 NHWC pixel-PAIR table:
    row r = [channels of pixel r, channels of pixel r+W] (1 KiB), so a
    single 2 KiB gather descriptor starting at row (y_low*W + x_low) fetches
    all 4 bilinear taps (y/y+1 x x/x+1) of a bin.

Per-core device program (SPMD, identical on all 8 cores):
  - bezier control points -> sample coords X, Y for all 32x1024 bins via
    PE matmuls against a constant basis matrix (layout [bin%128, g*32+n]),
    all in f32 (indices stay exact).
  - bilinear indices/weights via DVE elementwise ops; gather indices folded
    to dma_gather's 16-partition-wrapped int16 layout (identity-slice PE
    matmuls + permuted copies).
  - per ROI: ONE 1024-index dma_gather pulls 2 MiB of fp16 tap data;
    8 groups x 8 fp16 matmuls against diag(weight) matrices apply the 4
    bilinear terms and transpose [bin, ch] -> [ch, bin] into one f32 PSUM
    tile; 2 Act copies assemble fp16 [256ch, 1024bin]; 2 HWDGE DMAs write
    the ROI's NCHW output (fp16, upcast on host).
"""

import numpy as np

B, C, H, W = 8, 256, 128, 128
N_ROIS = 256
PH, PW = 16, 64
NB = PH * PW              # 1024 bins per ROI
NCORES = 8
R = 32                    # ROIs per core
SCALE = 0.25
PIX = H * W               # 16384 pixel rows per batch
TROWS = 2 * PIX           # table rows addressable by int16 idx (32768)
PADROWS = 192             # zero rows appended (x+1 overrun + idx clamp)
BUFROWS = TROWS + PADROWS
EW = 2 * C                # elems per pair-row (y and y+1 channel vectors)

_cache = {}


def _basis_const():
    """M[k, t]: X[n, t] = sum_k ctrl_x[n, k] * M[k, t], t = i*64 + j."""
    t = np.arange(NB)
    i, j = t // PW, t % PW
    u = j.astype(np.float64) / PW
    v = i.astype(np.float64) / PH
    co = [1.0, 3.0, 3.0, 1.0]
    M = np.zeros((8, NB), np.float64)
    for k in range(4):
        bern = co[k] * u**k * (1.0 - u) ** (3 - k)
        M[k] = SCALE * bern * (1.0 - v)       # top curve
        M[4 + k] = SCALE * bern * v           # bottom curve
    # Permute columns to the on-device (g, p) layout. Bin-at-position map
    # t(p, g) = (p%16)*64 + g*8 + p//16 makes DMA engine r (= position%16)
    # walk output row r along the curve -> near-contiguous gather addresses.
    gg = np.arange(8)[:, None]
    pp = np.arange(128)[None, :]
    tcol = (pp % 16) * 64 + gg * 8 + pp // 16
    return M[:, tcol.reshape(-1)].astype(np.float32)


def _build_program(rep=1):
    """rep>1 wraps the main ROI loop in a hardware repeat loop (benchmarking
    only — output is rewritten identically each iteration)."""
    import contextlib
    import concourse.bass as bass
    import concourse.bacc as bacc
    import concourse.tile as tile
    from concourse import mybir

    f32 = mybir.dt.float32
    f16 = mybir.dt.float16
    Alu = mybir.AluOpType
    Act = mybir.ActivationFunctionType

    nc = bacc.Bacc("TRN2", target_bir_lowering=False, debug=False)
    feat = nc.dram_tensor("feat", [BUFROWS, EW], f16, kind="ExternalInput")
    bez = nc.dram_tensor("bez", [R, 17], f32, kind="ExternalInput")
    mconst = nc.dram_tensor("mconst", [8, NB], f32, kind="ExternalInput")
    eye_d = nc.dram_tensor("eye", [128, 128], f32, kind="ExternalInput")
    eye16_d = nc.dram_tensor("eye16", [128, 128], f16, kind="ExternalInput")
    id32_d = nc.dram_tensor("id32", [32, 32], f32, kind="ExternalInput")
    c16k_d = nc.dram_tensor("c16k", [1, 128], f32, kind="ExternalInput")
    out_d = nc.dram_tensor("out", [R, C, PH, PW], f16, kind="ExternalOutput")

    # gather table view: idx unit = 1 pair-row (512 f16), elem = 2 pair-rows
    gather_src = bass.AP(feat[:].tensor, 0, [[EW, TROWS], [1, 2 * EW]])

    with tile.TileContext(nc) as tc:
        with (
            tc.tile_pool(name="const", bufs=1) as cpool,
            tc.tile_pool(name="work", bufs=1) as wpool,
        ):
            eye = cpool.tile([128, 128], f32)
            nc.sync.dma_start(eye[:], eye_d[:])
            eye16 = cpool.tile([128, 128], f16)
            nc.sync.dma_start(eye16[:], eye16_d[:])
            m_sb = cpool.tile([8, NB], f32)
            nc.sync.dma_start(m_sb[:], mconst[:])
            id32 = cpool.tile([32, 32], f32)
            nc.sync.dma_start(id32[:], id32_d[:])
            c16k = cpool.tile([1, 128], f32)
            nc.sync.dma_start(c16k[:], c16k_d[:])
            bez_sb = cpool.tile([R, 17], f32)
            nc.sync.dma_start(bez_sb[:], bez[:])

            w00 = wpool.tile([128, 256], f32)
            w01 = wpool.tile([128, 256], f32)
            w10 = wpool.tile([128, 256], f32)
            w11 = wpool.tile([128, 256], f32)
            widx = wpool.tile([128, R * 64], mybir.dt.int16)

            with (
                tc.tile_pool(name="setup", bufs=1) as spool,
                tc.tile_pool(name="psetup", bufs=1, space="PSUM") as pspool,
            ):
                # --- control points -> per-bin coords ------------------
                p_sep = spool.tile([R, 17], f32)
                nc.vector.tensor_copy(p_sep[:, 0:8], bez_sb[:, 1:17:2])
                nc.vector.tensor_copy(p_sep[:, 8:16], bez_sb[:, 2:17:2])
                nc.vector.tensor_copy(p_sep[:, 16:17], bez_sb[:, 0:1])
                pt_ps = pspool.tile([8, 3 * 32], f32)
                nc.tensor.transpose(out=pt_ps[0:8, 0:32], in_=p_sep[:, 0:8],
                                    identity=id32[:])
                nc.tensor.transpose(out=pt_ps[0:8, 32:64], in_=p_sep[:, 8:16],
                                    identity=id32[:])
                nc.tensor.transpose(out=pt_ps[0:1, 64:96], in_=p_sep[:, 16:17],
                                    identity=id32[:])
                pt = spool.tile([8, 3 * 32], f32)
                nc.vector.tensor_copy(pt[:, 0:64], pt_ps[0:8, 0:64])
                nc.vector.tensor_copy(pt[0:1, 64:96], pt_ps[0:1, 64:96])

                ps_x = pspool.tile([128, 256], f32)
                ps_y = pspool.tile([128, 256], f32)
                ps_b = pspool.tile([128, 256], f32)
                for g in range(8):
                    sl = slice(g * 32, (g + 1) * 32)
                    nc.tensor.matmul(out=ps_x[:, sl], lhsT=m_sb[:, g * 128:(g + 1) * 128],
                                     rhs=pt[0:8, 0:32], start=True, stop=True)
                    nc.tensor.matmul(out=ps_y[:, sl], lhsT=m_sb[:, g * 128:(g + 1) * 128],
                                     rhs=pt[0:8, 32:64], start=True, stop=True)
                    nc.tensor.matmul(out=ps_b[:, sl], lhsT=c16k[:],
                                     rhs=pt[0:1, 64:96], start=True, stop=True)

                # --- bilinear indices + weights ------------------------
                def T(name):
                    return spool.tile([128, 256], f32, name=name)

                xs, xl, lx, hx = T("xs"), T("xl"), T("lx"), T("hx")
                ys, yl, ly, hy = T("ys"), T("yl"), T("ly"), T("hy")
                tmp, val, tv = T("tmp"), T("val"), T("tv")
                ixf = T("ixf")

                v = nc.vector
                MAGIC = 12582912.0  # 1.5 * 2**23: (x+M)-M rounds x to nearest
                v.tensor_scalar(xs[:], ps_x[:], 0.0, None, Alu.max)
                v.tensor_scalar(xl[:], xs[:], MAGIC, -MAGIC, Alu.add, Alu.add)
                v.tensor_tensor(tmp[:], xl[:], xs[:], Alu.is_gt)
                v.tensor_tensor(xl[:], xl[:], tmp[:], Alu.subtract)
                v.tensor_scalar(xl[:], xl[:], float(W - 1), None, Alu.min)
                v.tensor_tensor(lx[:], xs[:], xl[:], Alu.subtract)
                v.tensor_scalar(tmp[:], xl[:], float(W - 1), None, Alu.is_lt)
                v.tensor_tensor(lx[:], lx[:], tmp[:], Alu.mult)
                v.tensor_scalar(hx[:], lx[:], 1.0, -1.0, Alu.subtract, Alu.mult)

                v.tensor_scalar(ys[:], ps_y[:], 0.0, None, Alu.max)
                v.tensor_scalar(yl[:], ys[:], MAGIC, -MAGIC, Alu.add, Alu.add)
                v.tensor_tensor(tmp[:], yl[:], ys[:], Alu.is_gt)
                v.tensor_tensor(yl[:], yl[:], tmp[:], Alu.subtract)
                v.tensor_scalar(yl[:], yl[:], float(H - 1), None, Alu.min)
                v.tensor_tensor(ly[:], ys[:], yl[:], Alu.subtract)
                v.tensor_scalar(tmp[:], yl[:], float(H - 1), None, Alu.is_lt)
                v.tensor_tensor(ly[:], ly[:], tmp[:], Alu.mult)
                v.tensor_scalar(hy[:], ly[:], 1.0, -1.0, Alu.subtract, Alu.mult)

                v.tensor_scalar(val[:], ps_x[:], float(W), None, Alu.is_lt)
                v.tensor_scalar(tv[:], ps_x[:], -1.0, None, Alu.is_gt)
                v.tensor_tensor(val[:], val[:], tv[:], Alu.mult)
                v.tensor_scalar(tv[:], ps_y[:], float(H), None, Alu.is_lt)
                v.tensor_tensor(val[:], val[:], tv[:], Alu.mult)
                v.tensor_scalar(tv[:], ps_y[:], -1.0, None, Alu.is_gt)
                v.tensor_tensor(val[:], val[:], tv[:], Alu.mult)
                v.tensor_tensor(hy[:], hy[:], val[:], Alu.mult)
                v.tensor_tensor(ly[:], ly[:], val[:], Alu.mult)

                v.tensor_tensor(w00[:], hy[:], hx[:], Alu.mult)
                v.tensor_tensor(w01[:], hy[:], lx[:], Alu.mult)
                v.tensor_tensor(w10[:], ly[:], hx[:], Alu.mult)
                v.tensor_tensor(w11[:], ly[:], lx[:], Alu.mult)

                v.tensor_scalar(ixf[:], yl[:], float(W), None, Alu.mult)
                v.tensor_tensor(ixf[:], ixf[:], xl[:], Alu.add)
                v.tensor_tensor(ixf[:], ixf[:], ps_b[:], Alu.add)

                # --- fold idx to dma_gather wrapped layout -------------
                # widx[r, n*64 + g*8 + q] = ix[q*16+r, g*32+n]
                fold = pspool.tile([16, 2048], f32, name="fold")
                for q in range(8):
                    nc.tensor.matmul(out=fold[:, q * 256:(q + 1) * 256],
                                     lhsT=eye[:, q * 16:(q + 1) * 16],
                                     rhs=ixf[:], start=True, stop=True)
                dst = (widx[0:16, :]
                       .rearrange("r (n g q) -> r n g q", n=R, g=8))
                perm = fold[:].rearrange("r (q g n) -> r n g q", q=8, g=8, n=32)
                nc.vector.tensor_copy(dst, perm)
                for k in range(1, 8):
                    nc.sync.dma_start(widx[16 * k:16 * (k + 1), :], widx[0:16, :])

            # --- main ROI loop ------------------------------------------
            with (
                tc.tile_pool(name="gath", bufs=4) as gpool,
                tc.tile_pool(name="stg", bufs=3) as stpool,
                tc.tile_pool(name="diag", bufs=3) as dpool,
                tc.tile_pool(name="pmain", bufs=2, space="PSUM") as ppool,
                tc.For_i(0, rep, 1) if rep > 1 else contextlib.nullcontext(),
            ):
                for n in range(R):
                    # one gather per ROI: elem = 2 pair-rows = all 4 taps,
                    # 2 KiB per descriptor. e = xoff*512 + k2*256 + ch
                    gab = gpool.tile([128, 8 * 1024], f16, name="gab")
                    nc.gpsimd.dma_gather(
                        out_ap=gab[:].rearrange("p (t e) -> p t e", e=1024),
                        in_ap=gather_src,
                        idxs_ap=widx[:, n * 64:(n + 1) * 64],
                        num_idxs=1024,
                        num_idxs_reg=1024,
                        elem_size=1024,
                        elem_step=512,
                    )
                    stage = stpool.tile([128, 2048], f16, name="stage")
                    po = ppool.tile([128, 2048], f32, name="po")
                    for g in range(8):
                        col = g * 32 + n
                        d00 = dpool.tile([128, 128], f16, name="d00")
                        d01 = dpool.tile([128, 128], f16, name="d01")
                        d10 = dpool.tile([128, 128], f16, name="d10")
                        d11 = dpool.tile([128, 128], f16, name="d11")
                        nc.vector.tensor_scalar(d00[:], eye16[:], w00[:, col:col + 1],
                                                None, Alu.mult)
                        nc.vector.tensor_scalar(d01[:], eye16[:], w01[:, col:col + 1],
                                                None, Alu.mult)
                        nc.vector.tensor_scalar(d10[:], eye16[:], w10[:, col:col + 1],
                                                None, Alu.mult)
                        nc.vector.tensor_scalar(d11[:], eye16[:], w11[:, col:col + 1],
                                                None, Alu.mult)
                        for h in range(2):
                            osl = slice(g * 256 + h * 128, g * 256 + (h + 1) * 128)
                            base = g * 1024 + h * 128
                            nc.tensor.matmul(out=po[:, osl], rhs=d00[:],
                                             lhsT=gab[:, base:base + 128],
                                             start=True, stop=False)
                            nc.tensor.matmul(out=po[:, osl], rhs=d10[:],
                                             lhsT=gab[:, base + 256:base + 384],
                                             start=False, stop=False)
                            nc.tensor.matmul(out=po[:, osl], rhs=d01[:],
                                             lhsT=gab[:, base + 512:base + 640],
                                             start=False, stop=False)
                            nc.tensor.matmul(out=po[:, osl], rhs=d11[:],
                                             lhsT=gab[:, base + 768:base + 896],
                                             start=False, stop=True)
                    # scatter psum (g, h, p) -> stage (h, bin t(p,g)): one
                    # strided copy per channel half h; Act and DVE drain the
                    # two halves in parallel so the PSUM tile frees sooner
                    for h in range(2):
                        src = (po[:].rearrange("c (g h2 a r) -> c h2 g a r",
                                               g=8, h2=2, a=8)[:, h])
                        dst = (stage[:, h * 1024:(h + 1) * 1024]
                               .rearrange("c (r g2 a) -> c g2 a r",
                                          r=16, g2=8, a=8))
                        if h == 0:
                            nc.scalar.activation(dst, src, Act.Copy)
                        else:
                            nc.vector.tensor_copy(dst, src)
                    for h in range(2):
                        nc.sync.dma_start(out_d[n, h * 128:(h + 1) * 128, :, :],
                                          stage[:, h * 1024:(h + 1) * 1024])

    nc.compile()
    return nc


def _route(batch):
    """Assign ROIs to cores: sorted by batch, each core spans <=2 adjacent
    batches, <=R ROIs. Returns (ids_per_core, base_per_core)."""
    order = np.argsort(batch, kind="stable")
    n = len(order)
    ids, bases = [], []
    i = 0
    for _ in range(NCORES):
        if i >= n:
            ids.append([])
            bases.append(0)
            continue
        base = int(batch[order[i]])
        cur = []
        while i < n and len(cur) < R and int(batch[order[i]]) <= base + 1:
            cur.append(int(order[i]))
            i += 1
        ids.append(cur)
        bases.append(base)
    if i < n:
        raise RuntimeError("ROI->core routing failed (batch distribution too "
                           "skewed for 8 cores x 2 batches)")
    return ids, bases


def kernel(input, beziers):
    from concourse.bass_utils import run_bass_kernel_spmd

    input = np.asarray(input, dtype=np.float32)
    beziers = np.asarray(beziers, dtype=np.float32)

    if "nc" not in _cache:
        _cache["nc"] = _build_program()
    nc = _cache["nc"]

    batch = beziers[:, 0].astype(np.int32)
    ids, bases = _route(batch)

    nhwc = np.ascontiguousarray(
        input.transpose(0, 2, 3, 1)).reshape(B * PIX, C).astype(np.float16)
    consts = {
        "mconst": _basis_const(),
        "eye": np.eye(128, dtype=np.float32),
        "eye16": np.eye(128, dtype=np.float16),
        "id32": np.eye(32, dtype=np.float32),
        "c16k": np.full((1, 128), float(PIX), np.float32),
    }

    in_maps = []
    for c in range(NCORES):
        buf = np.zeros((BUFROWS, EW), np.float16)
        lo = bases[c] * PIX
        hi = min((bases[c] + 2) * PIX, B * PIX)
        nrow = hi - lo
        buf[:nrow, :C] = nhwc[lo:hi]
        buf[:nrow - W, C:] = nhwc[lo + W:hi]          # pixel r+W channels
        bz = np.zeros((R, 17), np.float32)
        if ids[c]:
            rows = beziers[ids[c]].copy()
            rows[:, 0] = batch[ids[c]] - bases[c]
            bz[:len(ids[c])] = rows
        in_maps.append({"feat": buf, "bez": bz, **consts})

    _cache["in_maps"] = in_maps
    res = run_bass_kernel_spmd(nc, in_maps, list(range(NCORES)))

    out = np.zeros((N_ROIS, C, PH, PW), np.float32)
    for c in range(NCORES):
        if ids[c]:
            out[ids[c]] = res.results[c]["out"][:len(ids[c])].astype(np.float32)
    return out
